# revision 8
# baseline (speedup 1.0000x reference)
"""ComplexityAwareAttention Trainium2 Bass kernel (v2 schedule).

Sharding: 8 cores = 2 batches x 4 head-groups (3 heads each). Each core
computes q/k/v projections for its 3 heads, masked-key-gathered attention
(keys with attention_mask==0 are removed on host), and a partial output
projection (2048, 768). Host sums the 4 partials per batch and adds the
fused output bias (bo + Wo @ bv).

v2 schedule: the kernel is paced by ScalarE's exp stream (48 tiles of
[128,1024], ~1.15-1.33us each = the hard floor). The lead-in computes
only what the first QK needs (kT slot0 chunk0 + qT slot0 half0), so the
first exp fires at ~10us instead of ~49us. All remaining projection
units, the v-projection, and the half-0 output projection are interleaved
one-unit-per-step into the 48-step attention pipeline so the PE stays
dense (HAM stays warm) without ever starving the exp stream. Softmax
normalize is: DVE reciprocal_approx_fast on the PSUM denom row ->
Pool partition_broadcast -> one full-width DVE multiply.

PSUM budget (8 banks): sT double-buffer (4) + oacc (2) + projection
ping-pong (2).
"""

import math
import os
from contextlib import ExitStack

import numpy as np

import concourse.bass as bass
from concourse import bacc
import concourse.mybir as mybir
import concourse.tile as tile
from concourse.bass import ds, ts
from concourse.bass_utils import run_bass_kernel_spmd

F32 = mybir.dt.float32
F16 = mybir.dt.float16
AFT = mybir.ActivationFunctionType

B = 2
S = 2048
D = 768
H = 12
HD = 64
NH = 3  # heads per core
KT_D = D // 128  # 6 contraction tiles over d_model
SCORE_OFF = 12.5  # subtracted inside exp so et fits fp16 (scores reach ~22)

LAST_EXEC_TIME_NS = None
LAST_RESULTS = None


def build_nc(nk_t):
    n_k = nk_t * 128
    nkc = (n_k + 511) // 512  # xkT 512-col chunks
    nkp = nkc * 512  # padded key columns
    nc = bacc.Bacc(None, target_bir_lowering=False)

    # xT packed as (128, half, qc, kt, 512) so each (half, qc) projection
    # unit's DMA slice is contiguous per partition.
    d_xT = nc.dram_tensor("xT", (128, 2, 2, KT_D, 512), F16, kind="ExternalInput")
    d_xkT = nc.dram_tensor("xkT", (128, nkc, KT_D, 512), F16, kind="ExternalInput")
    d_wq = nc.dram_tensor("wq", (128, KT_D, 192), F16, kind="ExternalInput")
    d_wk = nc.dram_tensor("wk", (128, KT_D, 192), F16, kind="ExternalInput")
    d_wv = nc.dram_tensor("wv", (128, KT_D, 192), F16, kind="ExternalInput")
    d_wo = nc.dram_tensor("wo", (128, 2, D), F16, kind="ExternalInput")
    d_bq = nc.dram_tensor("bq", (128, 2), F32, kind="ExternalInput")
    d_bk = nc.dram_tensor("bk", (128, 2), F32, kind="ExternalInput")
    d_vcol = nc.dram_tensor("vcol", (128, nk_t), F16, kind="ExternalInput")
    d_out = nc.dram_tensor("out", (128, 16, D), F16, kind="ExternalOutput")

    with ExitStack() as ctx:
        tc = ctx.enter_context(tile.TileContext(nc))
        singles = ctx.enter_context(tc.tile_pool(name="singles", bufs=1))
        etp = ctx.enter_context(tc.tile_pool(name="etp", bufs=6))
        rowp = ctx.enter_context(tc.tile_pool(name="rowp", bufs=2))
        bcp = ctx.enter_context(tc.tile_pool(name="bcp", bufs=2))
        ogp = ctx.enter_context(tc.tile_pool(name="ogp", bufs=4))
        stp = ctx.enter_context(tc.tile_pool(name="stp", bufs=2, space="PSUM"))
        oap = ctx.enter_context(tc.tile_pool(name="oap", bufs=1, space="PSUM"))
        pjp = ctx.enter_context(tc.tile_pool(name="pjp", bufs=2, space="PSUM"))

        # Pull the Exp activation table load off the critical path.
        dummy = singles.tile([1, 2], F32)
        nc.vector.memset(dummy, 0.0)
        nc.scalar.activation(dummy, dummy, AFT.Exp)
        # per-partition exp bias (score offset; cancels in softmax)
        sb_soff = singles.tile([128, 1], F32)
        nc.vector.memset(sb_soff, -SCORE_OFF)

        sb_xT = singles.tile([128, 2, 2, KT_D, 512], F16)
        sb_xkT = singles.tile([128, nkc, KT_D, 512], F16)
        sb_wq = singles.tile([128, KT_D, 192], F16)
        sb_wk = singles.tile([128, KT_D, 192], F16)
        sb_wv = singles.tile([128, KT_D, 192], F16)
        sb_wo = singles.tile([128, 2, D], F16)
        sb_bq = singles.tile([128, 2], F32)
        sb_bk = singles.tile([128, 2], F32)
        sb_v = singles.tile([128, NH, nk_t, 65], F16)
        sb_qT = singles.tile([128, 2, S], F16)
        sb_kT = singles.tile([128, 2, nkp], F16)
        sb_onT = singles.tile([128, 2, S], F16)

        # ---- DMA: scalar HW queue = k path, sync HW queue = q path,
        # gpsimd SW queue = the rest. Emission order = need order.
        nc.scalar.dma_start(out=sb_wk, in_=d_wk[:, :, :])
        nc.scalar.dma_start(out=sb_xkT[:, 0], in_=d_xkT[:, 0])
        for h in range(NH):
            nc.scalar.dma_start(out=sb_v[:, h, :, 64:65], in_=d_vcol[:, :])
        for c in range(1, nkc):
            nc.scalar.dma_start(out=sb_xkT[:, c], in_=d_xkT[:, c])
        nc.sync.dma_start(out=sb_wq, in_=d_wq[:, :, :])
        nc.sync.dma_start(out=sb_bq, in_=d_bq[:, :])
        nc.sync.dma_start(out=sb_bk, in_=d_bk[:, :])
        nc.sync.dma_start(out=sb_xT[:, 0, 0], in_=d_xT[:, 0, 0])
        nc.sync.dma_start(out=sb_xT[:, 0, 1], in_=d_xT[:, 0, 1])
        nc.sync.dma_start(out=sb_xT[:, 1, 0], in_=d_xT[:, 1, 0])
        nc.sync.dma_start(out=sb_xT[:, 1, 1], in_=d_xT[:, 1, 1])
        nc.gpsimd.dma_start(out=sb_wv, in_=d_wv[:, :, :])
        nc.gpsimd.dma_start(out=sb_wo, in_=d_wo[:, :, :])

        # ---- projection / drain unit helpers ----
        def kproj(m, c):
            rows = 128 if m == 0 else 64
            msl = ds(m * 128, rows)
            ps = pjp.tile([128, 512], F32, tag="ps")
            for kt in range(KT_D):
                nc.tensor.matmul(
                    ps[:rows, :],
                    sb_wk[:, kt, msl],
                    sb_xkT[:, c, kt, :],
                    start=(kt == 0),
                    stop=(kt == KT_D - 1),
                )
            nc.vector.tensor_scalar_add(
                out=sb_kT[:rows, m, ds(c * 512, 512)],
                in0=ps[:rows, :],
                scalar1=sb_bk[:rows, m : m + 1],
            )

        def qproj(half, m, qc):
            rows = 128 if m == 0 else 64
            msl = ds(m * 128, rows)
            ps = pjp.tile([128, 512], F32, tag="ps")
            for kt in range(KT_D):
                nc.tensor.matmul(
                    ps[:rows, :],
                    sb_wq[:, kt, msl],
                    sb_xT[:, half, qc, kt, :],
                    start=(kt == 0),
                    stop=(kt == KT_D - 1),
                )
            nc.vector.tensor_scalar_add(
                out=sb_qT[:rows, m, ds(half * 1024 + qc * 512, 512)],
                in0=ps[:rows, :],
                scalar1=sb_bq[:rows, m : m + 1],
            )

        def vproj(kt2):
            c, off = (kt2 * 128) // 512, (kt2 * 128) % 512
            ps = pjp.tile([128, 512], F32, tag="ps")
            for kt in range(KT_D):
                nc.tensor.matmul(
                    ps[:, 0:192],
                    sb_xkT[:, c, kt, ds(off, 128)],
                    sb_wv[:, kt, :],
                    start=(kt == 0),
                    stop=(kt == KT_D - 1),
                )
            nc.vector.tensor_copy(
                out=sb_v[:, :, kt2, 0:64],
                in_=ps[:, 0:192].rearrange("p (h d) -> p h d", h=NH),
            )

        def oproj_tile(qt, og, j, cast_eng=None):
            # output projection for one 128-query tile; 2 e-chunks of
            # (512, 256) so each PSUM tile is a single bank.
            for eoff, ech in ((0, 512), (512, 256)):
                ps = pjp.tile([128, 512], F32, tag="ps")
                nc.tensor.matmul(
                    ps[:, :ech],
                    sb_onT[:, 0, ts(qt, 128)],
                    sb_wo[:, 0, ds(eoff, ech)],
                    start=True,
                    stop=False,
                )
                nc.tensor.matmul(
                    ps[:, :ech],
                    sb_onT[0:64, 1, ts(qt, 128)],
                    sb_wo[0:64, 1, ds(eoff, ech)],
                    start=False,
                    stop=True,
                )
                if cast_eng == "scalar":
                    nc.scalar.copy(out=og[:, j, ds(eoff, ech)], in_=ps[:, :ech])
                else:
                    nc.vector.tensor_copy(out=og[:, j, ds(eoff, ech)], in_=ps[:, :ech])

        # ---- attention step helpers ----
        def head_rows(head):
            # head 0 -> slot0 parts 0-63, head 1 -> slot0 parts 64-127,
            # head 2 -> slot1 parts 0-63
            qrow = 64 if head == 1 else 0
            slot = 1 if head == 2 else 0
            return qrow, slot

        def qk(head, half, kt2, sT):
            qrow, slot = head_rows(head)
            for qc in range(2):
                nc.tensor.matmul(
                    sT[:, ts(qc, 512)],
                    sb_kT[ds(qrow, 64), slot, ts(kt2, 128)],
                    sb_qT[ds(qrow, 64), slot, ds(half * 1024 + qc * 512, 512)],
                    start=True,
                    stop=True,
                )

        def pv(head, kt2, et, oacc):
            for qc in range(2):
                nc.tensor.matmul(
                    oacc[:, ts(qc, 512)],
                    sb_v[:, head, kt2, :],
                    et[:, ts(qc, 512)],
                    start=(kt2 == 0),
                    stop=(kt2 == nk_t - 1),
                )

        def norm(head, half, oacc):
            qrow, slot = head_rows(head)
            for ch in range(2):
                csl = ds(ch * 512, 512)
                drow = rowp.tile([1, 512], F32, tag="drow")
                nc.vector.tensor_copy(out=drow, in_=oacc[64:65, csl])
                rrow = rowp.tile([1, 512], F32, tag="rrow")
                nc.vector.reciprocal_approx_fast(out=rrow, in_=drow)
                rb = bcp.tile([64, 512], F32, tag="rb")
                nc.gpsimd.partition_broadcast(rb, rrow)
                nc.vector.tensor_mul(
                    out=sb_onT[ds(qrow, 64), slot, ds(half * 1024 + ch * 512, 512)],
                    in0=oacc[0:64, csl],
                    in1=rb,
                )

        # ---- the unit schedule: one deferred unit per attention step ----
        def oproj_pair(qt0, cast_eng=None):
            og = ogp.tile([128, 2, D], F16, tag="og")
            for j in range(2):
                oproj_tile(qt0 + j, og, j, cast_eng)
            nc.sync.dma_start(out=d_out[:, ds(qt0, 2), :], in_=og)

        units = {}  # step -> list of thunks
        # vproj(kt2) needed by PV(A, h0, kt2) at step kt2 (+1 pipeline lag)
        for kt2 in range(nk_t):
            units.setdefault(kt2, []).append(lambda k=kt2: vproj(k))
        # kproj chunks beyond c0 m0 (c1 needed at kt2=4; m1 before C-h0)
        for c in range(1, nkc):
            units.setdefault(2 * c - 1, []).append(lambda cc=c: kproj(0, cc))
        for c in range(nkc):
            units.setdefault(nk_t + 1 + 2 * c, []).append(lambda cc=c: kproj(1, cc))
        # qproj: h0 m1 (before C-h0 at step 2*nk_t), h1 m0 (before A-h1 at
        # 3*nk_t), h1 m1 (before C-h1 at 5*nk_t)
        units.setdefault(nk_t + 5, []).append(lambda: qproj(0, 1, 0))
        units.setdefault(nk_t + 7, []).append(lambda: qproj(0, 1, 1))
        units.setdefault(2 * nk_t + 1, []).append(lambda: qproj(1, 0, 0))
        units.setdefault(2 * nk_t + 3, []).append(lambda: qproj(1, 0, 1))
        units.setdefault(3 * nk_t + 1, []).append(lambda: qproj(1, 1, 0))
        units.setdefault(3 * nk_t + 3, []).append(lambda: qproj(1, 1, 1))
        # oproj for half 0 (q tiles 0-7): after norm(C, h0) which is
        # emitted at step 3*nk_t + 1. One pair every other step.
        for i in range(4):
            units.setdefault(3 * nk_t + 2 + 2 * i, []).append(
                lambda q=2 * i: oproj_pair(q)
            )

        # ---- lead-in: minimum to start (A, h0) ----
        kproj(0, 0)
        qproj(0, 0, 0)
        qproj(0, 0, 1)

        # ---- 48-step attention pipeline ----
        order = [(0, 0), (0, 1), (0, 2), (1, 0), (1, 1), (1, 2)]
        prev = None  # (head, half, kt2, et, oacc)
        step = 0
        for half, head in order:
            oacc = oap.tile([65, 1024], F32, tag="oacc")
            for kt2 in range(nk_t):
                sT = stp.tile([128, 1024], F32, tag="sT")
                qk(head, half, kt2, sT)
                et = etp.tile([128, 1024], F16, tag="et")
                nc.scalar.activation(et, sT, AFT.Exp, bias=sb_soff[:, 0:1])
                if prev is not None:
                    ph, phalf, pkt2, pet, poacc = prev
                    pv(ph, pkt2, pet, poacc)
                    if pkt2 == nk_t - 1:
                        norm(ph, phalf, poacc)
                for th in units.pop(step, []):
                    th()
                prev = (head, half, kt2, et, oacc)
                step += 1
        # flush last step
        ph, phalf, pkt2, pet, poacc = prev
        pv(ph, pkt2, pet, poacc)
        norm(ph, phalf, poacc)
        for s in sorted(units):
            for th in units.pop(s):
                th()

        # ---- tail: oproj for half 1 (q tiles 8-15); split casts
        # between ScalarE (now idle) and DVE.
        for i in range(4):
            oproj_pair(8 + 2 * i, cast_eng="scalar" if i % 2 == 0 else None)

    nc.compile()
    return nc


def kernel(
    hidden_states,
    complexity_scores,
    attention_mask,
    Wq,
    bq,
    Wk,
    bk,
    Wv,
    bv,
    Wo,
    bo,
    emb_table,
    comp_scaling,
):
    global LAST_EXEC_TIME_NS, LAST_RESULTS
    hs = np.asarray(hidden_states, np.float32)
    cs = np.asarray(complexity_scores).astype(np.int64)
    am = np.asarray(attention_mask)
    Wq = np.asarray(Wq, np.float32)
    bq = np.asarray(bq, np.float32)
    Wk = np.asarray(Wk, np.float32)
    bk = np.asarray(bk, np.float32)
    Wv = np.asarray(Wv, np.float32)
    bv = np.asarray(bv, np.float32)
    Wo = np.asarray(Wo, np.float32)
    bo = np.asarray(bo, np.float32)
    emb_table = np.asarray(emb_table, np.float32)
    comp_scaling = np.asarray(comp_scaling, np.float32)

    # per-head score scale (identical across batch: mean over batch of embs)
    embs = emb_table[cs]  # (B, H)
    scal = comp_scaling * embs.mean(axis=0)  # (H,)
    c = (scal / math.sqrt(HD)).astype(np.float32)

    # gather unmasked keys per batch; pad to a common multiple of 128
    idx = [np.nonzero(am[b] != 0)[0] for b in range(B)]
    n_max = max(1, max(len(i) for i in idx))
    nk_t = max(2, (n_max + 127) // 128)
    n_k = nk_t * 128
    nkc = (n_k + 511) // 512
    nkp = nkc * 512

    xT = []
    xkT = []
    vcol = []
    for b in range(B):
        t = hs[b].T.astype(np.float16)  # (768, 2048)
        # (128, half, qc, kt, 512)
        xT.append(
            np.ascontiguousarray(
                t.reshape(KT_D, 128, 2, 2, 512).transpose(1, 2, 3, 0, 4)
            )
        )
        tk = np.zeros((D, nkp), np.float16)
        tk[:, : len(idx[b])] = hs[b][idx[b]].T
        xkT.append(
            np.ascontiguousarray(
                tk.reshape(KT_D, 128, nkc, 512).transpose(1, 2, 0, 3)
            )
        )
        v = np.zeros((nk_t * 128,), np.float16)
        v[: len(idx[b])] = 1.0
        vcol.append(np.ascontiguousarray(v.reshape(nk_t, 128).T))

    WqT = Wq.T  # (d_in, e_out)
    WkT = Wk.T
    WvT = Wv.T
    WoT = np.ascontiguousarray(Wo.T)  # rows = attended feature d

    def pack_w(w192):  # (768, 192) -> (128, KT_D, 192)
        return np.ascontiguousarray(
            w192.astype(np.float16).reshape(KT_D, 128, 192).transpose(1, 0, 2)
        )

    def pack_bias(vec):  # (192,) -> (128, 2)
        out = np.zeros((128, 2), np.float32)
        out[:, 0] = vec[:128]
        out[:64, 1] = vec[128:]
        return out

    in_maps = []
    for core in range(8):
        b = core // 4
        heads = [3 * (core % 4) + j for j in range(NH)]
        cols = np.concatenate([np.arange(h * HD, (h + 1) * HD) for h in heads])
        cscale = np.repeat(c[heads], HD)  # (192,)
        wq_c = pack_w(WqT[:, cols] * cscale[None, :])
        bq_c = bq[cols] * cscale
        wk_c = pack_w(WkT[:, cols])
        bk_c = bk[cols]
        wv_c = pack_w(WvT[:, cols])
        wo_c = np.zeros((128, 2, D), np.float16)
        wo_c[:, 0, :] = WoT[cols[:128], :]
        wo_c[:64, 1, :] = WoT[cols[128:], :]
        in_maps.append(
            {
                "xT": xT[b],
                "xkT": xkT[b],
                "wq": wq_c,
                "wk": wk_c,
                "wv": wv_c,
                "wo": np.ascontiguousarray(wo_c),
                "bq": pack_bias(bq_c),
                "bk": pack_bias(bk_c),
                "vcol": vcol[b],
            }
        )

    nc = build_nc(nk_t)
    trace = os.environ.get("KERNEL_TRACE", "0") == "1"
    res = run_bass_kernel_spmd(nc, in_maps, core_ids=list(range(8)), trace=trace)
    LAST_EXEC_TIME_NS = res.exec_time_ns
    LAST_RESULTS = res

    bo_eff = (bo + Wo @ bv).astype(np.float64)
    out = np.empty((B, S, D), np.float32)
    for b in range(B):
        acc = np.zeros((S, D), np.float64)
        for g in range(4):
            p = res.results[4 * b + g]["out"]  # (128, 16, D) fp16
            acc += p.astype(np.float64).transpose(1, 0, 2).reshape(S, D)
        out[b] = (acc + bo_eff[None, :]).astype(np.float32)
    return out
